# revision 4
# baseline (speedup 1.0000x reference)
"""Trainium2 Bass kernel for nn_CompressiveMemory_57750130262084.

The reference computes (B=8, S=4096, DK=DV=1024):
    sigma  = elu(query) + 1                                  [B,S,DK]
    memory = einsum('bkd,bsv->bkv', swap(sigma), value)      [B,DK,DV]
    z_norm = sum_s sigma                                     [B,DK]
    out    = einsum('bsd,bkv->bsv', sigma, memory)
           / einsum('bsd,bk->bs',  sigma, z_norm)[..., None]

Every einsum uses disjoint summed subscripts, so each factorises into
outer products of independent reductions:
    memory[b,k,v]    = z_norm[b,k] * VS[b,v]      with VS[b,v] = sum_s value[b,s,v]
    retrieved[b,s,v] = rs[b,s] * Z[b] * VS[b,v]   with rs = rowsum(sigma), Z = sum_k z_norm
    denom[b,s]       = rs[b,s] * Z[b]
    out[b,s,v]       = VS[b,v]                    (exactly; query cancels)

So the complete per-batch result is the column-sum VS[b,:] of `value`
over S; every output row equals it. Sharding: data-parallel over batch,
one NeuronCore per batch element. The device kernel consumes the full
16 MB `value` shard and emits the [1, 1024] column-sum; the host-side
unshard gathers the 8 per-core rows and replicates them over S (the
rows are identical by construction, so replication is layout, not
compute). Device traffic is therefore the 16 MB input read, which is
the memory roofline for this reduction at the ~358 GB/s per-NC limit.

Schedule per core:
  - input as HWDGE DMAs of descending size (chunks of 128 rows x 1024
    cols). Chunk reductions are split ~2:1 between the DVE (fp32
    tensor_add chain into acc, ~1.23 us/chunk, capped at 1x mode) and
    the PE (PSUM-accumulating ones[128,128]^T @ chunk, 2 HW passes per
    f32 N=512 bank, ~2.2 us/chunk) so both trail the DMA stream. The
    DVE accumulator is folded into the same PSUM banks mid-stream, and
    the final chunks are PE-owned so the critical tail after the last
    input byte is ~2 passes + a [1,1024] copy + a 4 KB store.
"""

import numpy as np

B, S, D = 8, 4096, 1024
P = 128                 # SBUF partitions
N_CHUNK = S // P        # 32 row-chunks of 128 rows
IN_SIZES = [8, 8, 8, 4, 2, 1, 1]         # chunks per input DMA (sum = 32)
H = 512                 # PSUM bank width in f32 (matmul N limit)

_CACHE: dict = {}


def _build_program():
    import concourse.mybir as mybir
    import concourse.tile as tile
    from concourse import bacc

    assert sum(IN_SIZES) == N_CHUNK
    f32 = mybir.dt.float32
    nc = bacc.Bacc("TRN2", target_bir_lowering=False, debug=False, num_devices=B, enable_asserts=False)
    v = nc.declare_dram_parameter("value", [S, D], f32, isOutput=False)
    o = nc.declare_dram_parameter("out", [1, D], f32, isOutput=True)

    # Partition p owns rows [32p, 32p+32): a group of sz chunks then reads
    # ONE contiguous sz*4KB DRAM segment per partition (32KB descriptors
    # instead of 4KB), which is what the DMA engines need to hit peak
    # throughput. Row-to-partition assignment is irrelevant for a colsum.
    v_g = v[:].rearrange("(p n) m -> p n m", p=P)          # [128][32][1024]

    # Per-chunk reduction cost: DVE tensor_add ~1.23 us; PE (f32 matmul,
    # 2 HW passes per N=512 bank) ~2.2 us. Balance ~2:1 DVE:PE so both
    # trail the DMA stream. The last chunks go to the PE with the DVE-
    # accumulator fold emitted before them in PE queue order, so the
    # critical tail after the last input byte is just 2 PE passes + copy.
    pe_chunks = {c for c in range(N_CHUNK) if c % 3 == 2 and c < N_CHUNK - 2}
    pe_chunks |= {N_CHUNK - 2, N_CHUNK - 1}
    first_pe = min(pe_chunks)
    last_pe = N_CHUNK - 1
    fold_after = max(c for c in range(N_CHUNK) if c not in pe_chunks)  # last DVE chunk

    with tile.TileContext(nc) as tc:
        with (
            tc.tile_pool(name="in", bufs=1) as in_pool,
            tc.tile_pool(name="acc", bufs=1) as acc_pool,
            tc.tile_pool(name="ones", bufs=1) as ones_pool,
            tc.tile_pool(name="res", bufs=1) as res_pool,
            tc.tile_pool(name="psum", bufs=1, space="PSUM") as psum_pool,
        ):
            ones = ones_pool.tile([P, P], f32)
            nc.vector.memset(ones[:], 1.0)

            ps = psum_pool.tile([P, D], f32)
            acc = acc_pool.tile([P, D], f32)
            chunk0 = 0
            n_dve = 0
            for ti, sz in enumerate(IN_SIZES):
                t = in_pool.tile([P, sz * D], f32, tag=f"in{ti}")
                # DRAM side: per partition p, rows [32p+chunk0, 32p+chunk0+sz)
                src = v_g[:, chunk0 : chunk0 + sz]
                nc.sync.dma_start(t[:].rearrange("p (n m) -> p n m", n=sz), src)
                for n in range(sz):
                    c = chunk0 + n
                    sl = t[:, n * D : (n + 1) * D]
                    if c in pe_chunks:
                        for h in range(2):
                            nc.tensor.matmul(
                                ps[:, h * H : (h + 1) * H],
                                ones[:],
                                sl[:, h * H : (h + 1) * H],
                                start=(c == first_pe),
                                stop=(c == last_pe),
                            )
                    elif n_dve == 0:
                        nc.vector.tensor_copy(acc[:], sl)
                        n_dve += 1
                    else:
                        nc.vector.tensor_add(acc[:], acc[:], sl)
                        n_dve += 1
                    if c == fold_after:
                        # Fold the DVE accumulator into PSUM (mid-group).
                        for h in range(2):
                            nc.tensor.matmul(
                                ps[:, h * H : (h + 1) * H],
                                ones[:],
                                acc[:, h * H : (h + 1) * H],
                                start=False,
                                stop=False,
                            )
                chunk0 += sz

            # The ones^T matmul leaves the colsum replicated on all 128
            # PSUM partitions; one row is the result. Copy halves on two
            # engines to shorten the tail, then store 4 KB.
            res = res_pool.tile([1, D], f32)
            nc.vector.tensor_copy(res[:, 0:H], ps[0:1, 0:H])
            nc.scalar.copy(res[:, H:D], ps[0:1, H:D])
            nc.sync.dma_start(o[:], res[:])

    nc.compile()
    return nc


def _get_program():
    if "nc" not in _CACHE:
        _CACHE["nc"] = _build_program()
    return _CACHE["nc"]


def kernel(query: np.ndarray, value: np.ndarray) -> np.ndarray:
    from concourse.bass_utils import run_bass_kernel_spmd

    del query  # output is exactly independent of query (see module docstring)
    value = np.ascontiguousarray(value, dtype=np.float32)
    assert value.shape == (B, S, D)

    nc = _get_program()
    in_maps = [{"value": value[b]} for b in range(B)]
    try:
        res = run_bass_kernel_spmd(nc, in_maps, list(range(B)))
    except Exception:
        # The tunneled runtime occasionally surfaces a transient
        # NRT_EXEC_UNIT_UNRECOVERABLE on the first dispatch; retry once.
        import time

        time.sleep(2.0)
        res = run_bass_kernel_spmd(nc, in_maps, list(range(B)))
    vs = np.stack([res.results[b]["out"][0] for b in range(B)], axis=0)  # [B, D]
    # Unshard: every output row of batch b equals VS[b,:] (see docstring).
    out = np.empty((B, S, D), dtype=np.float32)
    out[:] = vs[:, None, :]
    return out
